# revision 18
# baseline (speedup 1.0000x reference)
"""Trainium2 Bass kernel for the nn_Aggregate GNN message-passing problem.

Computation (see reference):
    keep = (A > 0) limited to the first `neibor_num` set entries per row
    nb_mean = (keep @ X) / max(cnt, 1)
    out = leaky_relu(X @ W_line.T + b_line)
        + where(cnt > 0, leaky_relu(nb_mean @ W_nb.T + b_nb), 0)

Sharding: rows of A / output rows are split across 8 cores (1024 rows each).
No collectives are needed: each core gets its A row-block (transposed), its
X row-block (transposed), the shared X head rows, and the weights.

Key structural fact exploited: `keep` zeroes every set bit after the
`neibor_num`-th, so only the first C columns of A can contribute, where C
bounds the column position of the nn-th set bit over all rows.  The host
verifies exactly (cheaply) that every row reaches `neibor_num` set bits
within the first C=256 columns; in that case cnt == neibor_num for every
row and the kernel contracts over 256 neighbor candidates instead of 8192.
If the check fails (it cannot for the target input distribution), a numpy
fallback computes the exact reference semantics.

Device pipeline per core (rows R=1024, C=256, Cin=Cout=512):
  1. mask:    mbT[j, r] = (A[r, j] > 0) via DVE is_gt on the transposed
              A block (uint8 -> fp16 0/1; exact).
  2. prefix:  cumT = LTRI.T @ mbT per 128-column chunk (+ ONES.T @ mbT of
              earlier chunks) gives the inclusive prefix count of set bits
              along the row, in transposed layout, on the PE (fp16 inputs,
              fp32 accumulation; counts <= 256 so exact).
  3. keepT = (cumT <= nn) * mbT                      (one fused DVE op)
  4. nb_sumT = X_head.T-contract keepT               (PE)
  5. xj = leaky(nb_sumT.T @ (W_nb.T/nn) + b_nb)      (PE + ACT Lrelu)
     xi = leaky(X_blk @ W_line.T + b_line)           (PE + ACT Lrelu)
     out = xi + xj                                   (DVE, fp32)
Biases are added with k=1 matmuls (ones-row x bias-row) into the same PSUM
accumulation group.

Matmul operands are fp16 (e5m10): 16-bit weights enable the PE's fast
weight load / background weight buffer, so LDWEIGHTS hides behind the
matmul stream (4-byte fp32r weights serialize, ~2x slower per matmul).
Accumulation is always fp32 in PSUM; masks and counts are exact in fp16.

DMA strategy: each logical input is packed on the host into one wide
[128, *] (or [1, *]) tensor and loaded with a single DMA (descriptor issue
costs ~0.7us each, so many small loads serialize the startup).  The
latency-critical tensors (at/smalls/xh/row-consts) ride the HW DGE
(nc.sync); bulk stage-2 operands (xt, weights) ride the SW DGE (nc.gpsimd).
"""

import numpy as np

NCORES = 8
N = 8192
CIN = 512
COUT = 512
R = N // NCORES          # rows per core
C = 256                  # neighbor-candidate column window
KC = C // 128            # 128-col chunks of the window
MC = CIN // 128          # 128-row chunks of the feature dim
RT = R // 128            # 128-row output tiles per core
NEG_SLOPE = 0.01         # jax.nn.leaky_relu default

_nc_cache = {}
LAST_RESULT = None       # BassKernelResults of the most recent device run
SIM_SAFE = False         # CoreSim lacks Lrelu; True swaps in a Relu decomposition


def _build_nc(nn: int):
    import concourse.bass as bass
    import concourse.bacc as bacc
    import concourse.mybir as mybir
    import concourse.tile as tile

    F32 = mybir.dt.float32
    BF16 = mybir.dt.bfloat16  # PE fast path for the mask/xj side
    F32R = mybir.dt.float32r  # single-pass FP22; used on the precision-critical xi path
    U8 = mybir.dt.uint8
    AF = mybir.ActivationFunctionType
    OP = mybir.AluOpType

    nc = bacc.Bacc("TRN2", target_bir_lowering=False, debug=False)

    at_d = nc.dram_tensor("at", [128, KC * R], U8, kind="ExternalInput")
    xh_d = nc.dram_tensor("xh", [128, KC * CIN], BF16, kind="ExternalInput")
    xt_d = nc.dram_tensor("xt", [128, MC * R], F32R, kind="ExternalInput")
    wnbt_d = nc.dram_tensor("wnbt", [128, MC * COUT], BF16, kind="ExternalInput")
    wlt_d = nc.dram_tensor("wlt", [128, MC * COUT], F32R, kind="ExternalInput")
    sm_d = nc.dram_tensor("sm", [128, 256], BF16, kind="ExternalInput")
    rcb_d = nc.dram_tensor("rcb", [1, COUT + 128], BF16, kind="ExternalInput")
    rcf_d = nc.dram_tensor("rcf", [1, COUT + 128], F32R, kind="ExternalInput")
    out_d = nc.dram_tensor("out", [R, COUT], F32, kind="ExternalOutput")

    with tile.TileContext(nc) as tc:
        with (
            tc.tile_pool(name="const", bufs=1) as constp,
            tc.tile_pool(name="mask", bufs=1) as maskp,
            tc.tile_pool(name="work", bufs=3) as workp,
            tc.tile_pool(name="psum", bufs=2, space=bass.MemorySpace.PSUM) as psump,
        ):
            # --- latency-critical loads (HW DGE), smallest-first ---------
            sm = constp.tile([128, 256], BF16, name="sm_sb")
            nc.sync.dma_start(sm[:], sm_d[:])
            at_sb = maskp.tile([128, KC * R], U8, name="at_sb")
            for t in range(KC):
                nc.sync.dma_start(at_sb[:, t * R:(t + 1) * R],
                                  at_d[:, t * R:(t + 1) * R])
            xh_sb = constp.tile([128, KC * CIN], BF16, name="xh_sb")
            nc.sync.dma_start(xh_sb[:], xh_d[:])
            rcb = constp.tile([1, COUT + 128], BF16, name="rcb_sb")
            nc.sync.dma_start(rcb[:], rcb_d[:])
            rcf = constp.tile([1, COUT + 128], F32R, name="rcf_sb")
            nc.sync.dma_start(rcf[:], rcf_d[:])

            ltri = sm[:, 0:128]
            ones = sm[:, 128:256]
            bnb = rcb[:, 0:COUT]
            onesb = rcb[:, COUT:]
            bl = rcf[:, 0:COUT]
            onesf = rcf[:, COUT:]
            at = [at_sb[:, t * R:(t + 1) * R] for t in range(KC)]
            xh = [xh_sb[:, t * CIN:(t + 1) * CIN] for t in range(KC)]

            # --- bulk stage-2 operands (SW DGE, overlap with mask path) --
            xt_sb = constp.tile([128, MC * R], F32R, name="xt_sb")
            nc.gpsimd.dma_start(xt_sb[:], xt_d[:])
            wlt_sb = constp.tile([128, MC * COUT], F32R, name="wlt_sb")
            nc.gpsimd.dma_start(wlt_sb[:], wlt_d[:])
            wnbt_sb = constp.tile([128, MC * COUT], BF16, name="wnbt_sb")
            nc.gpsimd.dma_start(wnbt_sb[:], wnbt_d[:])
            xt = [xt_sb[:, m * R:(m + 1) * R] for m in range(MC)]
            wnbt = [wnbt_sb[:, m * COUT:(m + 1) * COUT] for m in range(MC)]
            wlt = [wlt_sb[:, m * COUT:(m + 1) * COUT] for m in range(MC)]

            # 1. A block -> fp16 0/1 mask, transposed layout [col, row]
            mb = []
            for t in range(KC):
                mb_t = maskp.tile([128, R], BF16, name=f"mb{t}")
                nc.vector.tensor_scalar(mb_t[:], at[t], 0, None, op0=OP.is_gt)
                mb.append(mb_t)

            # 2+3. prefix count along the row (PE) -> keep mask (DVE)
            keep = []
            for t in range(KC):
                keep_t = maskp.tile([128, R], BF16, name=f"keep{t}")
                keep.append(keep_t)
            for t in range(KC):
                for h in range(R // 512):
                    sl = slice(h * 512, (h + 1) * 512)
                    cum = psump.tile([128, 512], F32, name="cum")
                    for s in range(t + 1):
                        nc.tensor.matmul(
                            cum[:],
                            ltri if s == t else ones,
                            mb[s][:, sl],
                            start=(s == 0),
                            stop=(s == t),
                        )
                    # keep = (cum <= nn) * mb
                    nc.vector.scalar_tensor_tensor(
                        keep[t][:, sl], cum[:], float(nn), mb[t][:, sl],
                        op0=OP.is_le, op1=OP.mult,
                    )

            # 4. nb_sumT[c, r] = sum_k X[k, c] * keep[k, r]  (the pre-scaled
            #    weights absorb the 1/nn mean factor)
            nbm = []
            for m in range(MC):
                nbm_m = maskp.tile([128, R], BF16, name=f"nbm{m}")
                nbm.append(nbm_m)
            for m in range(MC):
                for h in range(R // 512):
                    sl = slice(h * 512, (h + 1) * 512)
                    ps = psump.tile([128, 512], F32, name="psnb")
                    for t in range(KC):
                        nc.tensor.matmul(
                            ps[:],
                            xh[t][:, m * 128:(m + 1) * 128],
                            keep[t][:, sl],
                            start=(t == 0),
                            stop=(t == KC - 1),
                        )
                    # PSUM -> SBUF copies split between ACT and DVE
                    if (m * 2 + h) % 2 == 0:
                        nc.scalar.activation(nbm[m][:, sl], ps[:], AF.Copy)
                    else:
                        nc.vector.tensor_copy(nbm[m][:, sl], ps[:])

            # 5. two linears + leaky relu + add, per 128-row output tile
            def leaky(ps, out_sb):
                if SIM_SAFE:
                    t = workp.tile([128, COUT], F32, name="lrt")
                    nc.scalar.activation(t[:], ps[:], AF.Relu,
                                         scale=1.0 - NEG_SLOPE)
                    nc.vector.scalar_tensor_tensor(
                        out_sb[:], ps[:], NEG_SLOPE, t[:],
                        op0=OP.mult, op1=OP.add)
                else:
                    nc.scalar.activation(out_sb[:], ps[:], AF.Lrelu,
                                         alpha=NEG_SLOPE)

            for r in range(RT):
                rsl = slice(r * 128, (r + 1) * 128)
                psi = psump.tile([128, COUT], F32, name="psi")
                for m in range(MC):
                    nc.tensor.matmul(
                        psi[:], xt[m][:, rsl], wlt[m],
                        start=(m == 0), stop=False,
                    )
                nc.tensor.matmul(psi[:], onesf, bl, start=False, stop=True)
                xi = workp.tile([128, COUT], F32, name="xi")
                leaky(psi, xi)

                psj = psump.tile([128, COUT], F32, name="psj")
                for m in range(MC):
                    nc.tensor.matmul(
                        psj[:], nbm[m][:, rsl], wnbt[m],
                        start=(m == 0), stop=False,
                    )
                nc.tensor.matmul(psj[:], onesb, bnb, start=False, stop=True)
                xj = workp.tile([128, COUT], F32, name="xj")
                leaky(psj, xj)

                ot = workp.tile([128, COUT], F32, name="ot")
                nc.vector.tensor_tensor(ot[:], xi[:], xj[:], op=OP.add)
                nc.sync.dma_start(out_d[rsl, :], ot[:])

    nc.compile()
    return nc


def _get_nc(nn: int):
    if nn not in _nc_cache:
        _nc_cache[nn] = _build_nc(nn)
    return _nc_cache[nn]


def _numpy_fallback(X, A, W_nb, b_nb, W_line, b_line, nn):
    def leaky(x):
        return np.where(x >= 0, x, NEG_SLOPE * x)

    Ab = A > 0
    keep = Ab & (np.cumsum(Ab.astype(np.int64), axis=1) <= nn)
    cnt = keep.sum(axis=1, keepdims=True).astype(X.dtype)
    nb_sum = keep.astype(X.dtype) @ X
    nb_mean = nb_sum / np.maximum(cnt, 1.0)
    xj = leaky(nb_mean @ W_nb.T + b_nb)
    xi = leaky(X @ W_line.T + b_line)
    return (xi + np.where(cnt > 0, xj, 0.0)).astype(np.float32)


def _pack128(arr):
    """[128*k, m] -> [128, k*m] with block i in columns [i*m:(i+1)*m]."""
    k = arr.shape[0] // 128
    return np.ascontiguousarray(
        arr.reshape(k, 128, arr.shape[1]).transpose(1, 0, 2).reshape(128, -1))


def build_in_maps(X, A, W_nb, b_nb, W_line, b_line, nn):
    """Shard the full inputs into one input map per core."""
    import ml_dtypes
    bf = ml_dtypes.bfloat16
    ATall = np.ascontiguousarray((A[:, :C] > 0).T.astype(np.uint8))  # [C, N]
    XTall = np.ascontiguousarray(X.T)                                # [CIN, N]
    xh = _pack128(X[:C, :].astype(bf))                               # [128, KC*CIN]
    wnbt = _pack128(np.ascontiguousarray(W_nb.T.astype(np.float32)
                                         * np.float32(1.0 / nn)).astype(bf))
    wlt = _pack128(np.ascontiguousarray(W_line.T))
    sm = np.concatenate([np.triu(np.ones((128, 128), bf)),
                         np.ones((128, 128), bf)], axis=1)   # [128, 256]
    rcb = np.concatenate([b_nb.astype(bf).reshape(1, COUT),
                          np.ones((1, 128), bf)], axis=1)
    rcf = np.concatenate([b_line.astype(np.float32).reshape(1, COUT),
                          np.ones((1, 128), np.float32)], axis=1)
    in_maps = []
    for c in range(NCORES):
        rows = slice(c * R, (c + 1) * R)
        in_maps.append({
            "at": _pack128(ATall[:, rows]),
            "xh": xh,
            "xt": _pack128(XTall[:, rows]),
            "wnbt": wnbt,
            "wlt": wlt,
            "sm": sm,
            "rcb": rcb,
            "rcf": rcf,
        })
    return in_maps


def kernel(**inputs) -> np.ndarray:
    global LAST_RESULT
    X = np.ascontiguousarray(np.asarray(inputs["X"], dtype=np.float32))
    A = np.ascontiguousarray(np.asarray(inputs["A"], dtype=np.int32))
    W_nb = np.asarray(inputs["W_nb"], dtype=np.float32)
    b_nb = np.asarray(inputs["b_nb"], dtype=np.float32)
    W_line = np.asarray(inputs["W_line"], dtype=np.float32)
    b_line = np.asarray(inputs["b_line"], dtype=np.float32)
    nn = int(np.asarray(inputs["neibor_num"]))

    # Fast path requires: every row reaches nn set bits within the first C
    # columns (=> keep-mask confined to [:, :C] and cnt == nn > 0 per row).
    fast = (
        X.shape == (N, CIN) and A.shape == (N, N) and 1 <= nn <= C
        and int(np.count_nonzero(A[:, :C] > 0, axis=1).min()) >= nn
    )
    if not fast:
        return _numpy_fallback(X, A, W_nb, b_nb, W_line, b_line, nn)

    from concourse.bass_utils import run_bass_kernel_spmd

    in_maps = build_in_maps(X, A, W_nb, b_nb, W_line, b_line, nn)
    nc = _get_nc(nn)
    res = run_bass_kernel_spmd(nc, in_maps, core_ids=list(range(NCORES)))
    LAST_RESULT = res
    return np.concatenate([r["out"] for r in res.results], axis=0)


if __name__ == "__main__":
    rng = np.random.default_rng(0)
    X = rng.standard_normal((N, CIN), dtype=np.float32)
    A = (rng.random((N, N)) < 0.5).astype(np.int32)
    W_nb = rng.standard_normal((COUT, CIN), dtype=np.float32) * 0.04
    b_nb = rng.standard_normal(COUT, dtype=np.float32) * 0.04
    W_line = rng.standard_normal((COUT, CIN), dtype=np.float32) * 0.04
    b_line = rng.standard_normal(COUT, dtype=np.float32) * 0.04
    out = kernel(X=X, A=A, W_nb=W_nb, b_nb=b_nb, W_line=W_line,
                 b_line=b_line, neibor_num=64)
    exp = _numpy_fallback(X, A, W_nb, b_nb, W_line, b_line, 64)
    err = np.abs(out - exp).max() / np.abs(exp).max()
    print("self-test rel err:", err)


# revision 19
# speedup vs baseline: 1.0962x; 1.0962x over previous
"""Trainium2 Bass kernel for the nn_Aggregate GNN message-passing problem.

Computation (see reference):
    keep = (A > 0) limited to the first `neibor_num` set entries per row
    nb_mean = (keep @ X) / max(cnt, 1)
    out = leaky_relu(X @ W_line.T + b_line)
        + where(cnt > 0, leaky_relu(nb_mean @ W_nb.T + b_nb), 0)

Sharding: rows of A / output rows are split across 8 cores (1024 rows each).
No collectives are needed: each core gets its A row-block (transposed), its
X row-block (transposed), the shared X head rows, and the weights.

Key structural fact exploited: `keep` zeroes every set bit after the
`neibor_num`-th, so only the first C columns of A can contribute, where C
bounds the column position of the nn-th set bit over all rows.  The host
verifies exactly (cheaply) that every row reaches `neibor_num` set bits
within the first C=256 columns; in that case cnt == neibor_num for every
row and the kernel contracts over 256 neighbor candidates instead of 8192.
If the check fails (it cannot for the target input distribution), a numpy
fallback computes the exact reference semantics.

Device pipeline per core (rows R=1024, C=256, Cin=Cout=512):
  1. mask:    mbT[j, r] = (A[r, j] > 0) via DVE is_gt on the transposed
              A block (uint8 -> fp16 0/1; exact).
  2. prefix:  cumT = LTRI.T @ mbT per 128-column chunk (+ ONES.T @ mbT of
              earlier chunks) gives the inclusive prefix count of set bits
              along the row, in transposed layout, on the PE (fp16 inputs,
              fp32 accumulation; counts <= 256 so exact).
  3. keepT = (cumT <= nn) * mbT                      (one fused DVE op)
  4. nb_sumT = X_head.T-contract keepT               (PE)
  5. xj = leaky(nb_sumT.T @ (W_nb.T/nn) + b_nb)      (PE + ACT Lrelu)
     xi = leaky(X_blk @ W_line.T + b_line)           (PE + ACT Lrelu)
     out = xi + xj                                   (DVE, fp32)
Biases are added with k=1 matmuls (ones-row x bias-row) into the same PSUM
accumulation group.

Matmul operands are fp16 (e5m10): 16-bit weights enable the PE's fast
weight load / background weight buffer, so LDWEIGHTS hides behind the
matmul stream (4-byte fp32r weights serialize, ~2x slower per matmul).
Accumulation is always fp32 in PSUM; masks and counts are exact in fp16.

DMA strategy: each logical input is packed on the host into one wide
[128, *] (or [1, *]) tensor and loaded with a single DMA (descriptor issue
costs ~0.7us each, so many small loads serialize the startup).  The
latency-critical tensors (at/smalls/xh/row-consts) ride the HW DGE
(nc.sync); bulk stage-2 operands (xt, weights) ride the SW DGE (nc.gpsimd).
"""

import numpy as np

NCORES = 8
N = 8192
CIN = 512
COUT = 512
R = N // NCORES          # rows per core
C = 256                  # neighbor-candidate column window
KC = C // 128            # 128-col chunks of the window
MC = CIN // 128          # 128-row chunks of the feature dim
RT = R // 128            # 128-row output tiles per core
NEG_SLOPE = 0.01         # jax.nn.leaky_relu default

_nc_cache = {}
LAST_RESULT = None       # BassKernelResults of the most recent device run
SIM_SAFE = False         # CoreSim lacks Lrelu; True swaps in a Relu decomposition


def _build_nc(nn: int):
    import concourse.bass as bass
    import concourse.bacc as bacc
    import concourse.mybir as mybir
    import concourse.tile as tile

    F32 = mybir.dt.float32
    BF16 = mybir.dt.bfloat16  # PE fast path for the mask/xj side
    FP16 = mybir.dt.float16   # e5m10 for the precision-critical xi path
    U8 = mybir.dt.uint8
    AF = mybir.ActivationFunctionType
    OP = mybir.AluOpType

    nc = bacc.Bacc("TRN2", target_bir_lowering=False, debug=False)

    at_d = nc.dram_tensor("at", [128, KC * R], U8, kind="ExternalInput")
    xh_d = nc.dram_tensor("xh", [128, KC * CIN], BF16, kind="ExternalInput")
    xt_d = nc.dram_tensor("xt", [128, MC * R], FP16, kind="ExternalInput")
    wnbt_d = nc.dram_tensor("wnbt", [128, MC * COUT], BF16, kind="ExternalInput")
    wlt_d = nc.dram_tensor("wlt", [128, MC * COUT], FP16, kind="ExternalInput")
    sm_d = nc.dram_tensor("sm", [128, 256], BF16, kind="ExternalInput")
    rcb_d = nc.dram_tensor("rcb", [1, COUT + 128], BF16, kind="ExternalInput")
    rcf_d = nc.dram_tensor("rcf", [1, COUT + 128], FP16, kind="ExternalInput")
    out_d = nc.dram_tensor("out", [R, COUT], F32, kind="ExternalOutput")

    with tile.TileContext(nc) as tc:
        with (
            tc.tile_pool(name="const", bufs=1) as constp,
            tc.tile_pool(name="mask", bufs=1) as maskp,
            tc.tile_pool(name="work", bufs=3) as workp,
            tc.tile_pool(name="psum", bufs=2, space=bass.MemorySpace.PSUM) as psump,
        ):
            # --- latency-critical loads (HW DGE): at gates everything ----
            at_sb = maskp.tile([128, KC * R], U8, name="at_sb")
            nc.sync.dma_start(at_sb[:], at_d[:])
            sm = constp.tile([128, 256], BF16, name="sm_sb")
            nc.sync.dma_start(sm[:], sm_d[:])
            xh_sb = constp.tile([128, KC * CIN], BF16, name="xh_sb")
            nc.sync.dma_start(xh_sb[:], xh_d[:])
            rcb = constp.tile([1, COUT + 128], BF16, name="rcb_sb")
            nc.sync.dma_start(rcb[:], rcb_d[:])
            rcf = constp.tile([1, COUT + 128], FP16, name="rcf_sb")
            nc.sync.dma_start(rcf[:], rcf_d[:])

            ltri = sm[:, 0:128]
            ones = sm[:, 128:256]
            bnb = rcb[:, 0:COUT]
            onesb = rcb[:, COUT:]
            bl = rcf[:, 0:COUT]
            onesf = rcf[:, COUT:]
            at = [at_sb[:, t * R:(t + 1) * R] for t in range(KC)]
            xh = [xh_sb[:, t * CIN:(t + 1) * CIN] for t in range(KC)]

            # --- bulk stage-2 operands (SW DGE, overlap with mask path) --
            xt_sb = constp.tile([128, MC * R], FP16, name="xt_sb")
            nc.gpsimd.dma_start(xt_sb[:], xt_d[:])
            wlt_sb = constp.tile([128, MC * COUT], FP16, name="wlt_sb")
            nc.gpsimd.dma_start(wlt_sb[:], wlt_d[:])
            wnbt_sb = constp.tile([128, MC * COUT], BF16, name="wnbt_sb")
            nc.gpsimd.dma_start(wnbt_sb[:], wnbt_d[:])
            xt = [xt_sb[:, m * R:(m + 1) * R] for m in range(MC)]
            wnbt = [wnbt_sb[:, m * COUT:(m + 1) * COUT] for m in range(MC)]
            wlt = [wlt_sb[:, m * COUT:(m + 1) * COUT] for m in range(MC)]

            # 1. A block -> fp16 0/1 mask, transposed layout [col, row]
            mb = []
            for t in range(KC):
                mb_t = maskp.tile([128, R], BF16, name=f"mb{t}")
                nc.vector.tensor_scalar(mb_t[:], at[t], 0, None, op0=OP.is_gt)
                mb.append(mb_t)

            # 2+3. prefix count along the row (PE) -> keep mask (DVE)
            keep = []
            for t in range(KC):
                keep_t = maskp.tile([128, R], BF16, name=f"keep{t}")
                keep.append(keep_t)
            for t in range(KC):
                for h in range(R // 512):
                    sl = slice(h * 512, (h + 1) * 512)
                    cum = psump.tile([128, 512], F32, name="cum")
                    for s in range(t + 1):
                        nc.tensor.matmul(
                            cum[:],
                            ltri if s == t else ones,
                            mb[s][:, sl],
                            start=(s == 0),
                            stop=(s == t),
                        )
                    # keep = (cum <= nn) * mb
                    nc.vector.scalar_tensor_tensor(
                        keep[t][:, sl], cum[:], float(nn), mb[t][:, sl],
                        op0=OP.is_le, op1=OP.mult,
                    )

            # 4. nb_sumT[c, r] = sum_k X[k, c] * keep[k, r]  (the pre-scaled
            #    weights absorb the 1/nn mean factor)
            nbm = []
            for m in range(MC):
                nbm_m = maskp.tile([128, R], BF16, name=f"nbm{m}")
                nbm.append(nbm_m)
            for m in range(MC):
                for h in range(R // 512):
                    sl = slice(h * 512, (h + 1) * 512)
                    ps = psump.tile([128, 512], F32, name="psnb")
                    for t in range(KC):
                        nc.tensor.matmul(
                            ps[:],
                            xh[t][:, m * 128:(m + 1) * 128],
                            keep[t][:, sl],
                            start=(t == 0),
                            stop=(t == KC - 1),
                        )
                    # PSUM -> SBUF copies split between ACT and DVE
                    if (m * 2 + h) % 2 == 0:
                        nc.scalar.activation(nbm[m][:, sl], ps[:], AF.Copy)
                    else:
                        nc.vector.tensor_copy(nbm[m][:, sl], ps[:])

            # 5. two linears + leaky relu + add, per 128-row output tile
            def leaky(ps, out_sb):
                if SIM_SAFE:
                    t = workp.tile([128, COUT], F32, name="lrt")
                    nc.scalar.activation(t[:], ps[:], AF.Relu,
                                         scale=1.0 - NEG_SLOPE)
                    nc.vector.scalar_tensor_tensor(
                        out_sb[:], ps[:], NEG_SLOPE, t[:],
                        op0=OP.mult, op1=OP.add)
                else:
                    nc.scalar.activation(out_sb[:], ps[:], AF.Lrelu,
                                         alpha=NEG_SLOPE)

            for r in range(RT):
                rsl = slice(r * 128, (r + 1) * 128)
                psi = psump.tile([128, COUT], F32, name="psi")
                for m in range(MC):
                    nc.tensor.matmul(
                        psi[:], xt[m][:, rsl], wlt[m],
                        start=(m == 0), stop=False,
                    )
                nc.tensor.matmul(psi[:], onesf, bl, start=False, stop=True)
                xi = workp.tile([128, COUT], F32, name="xi")
                leaky(psi, xi)

                psj = psump.tile([128, COUT], F32, name="psj")
                for m in range(MC):
                    nc.tensor.matmul(
                        psj[:], nbm[m][:, rsl], wnbt[m],
                        start=(m == 0), stop=False,
                    )
                nc.tensor.matmul(psj[:], onesb, bnb, start=False, stop=True)
                xj = workp.tile([128, COUT], F32, name="xj")
                leaky(psj, xj)

                ot = workp.tile([128, COUT], F32, name="ot")
                nc.vector.tensor_tensor(ot[:], xi[:], xj[:], op=OP.add)
                nc.sync.dma_start(out_d[rsl, :], ot[:])

    nc.compile()
    return nc


def _get_nc(nn: int):
    if nn not in _nc_cache:
        _nc_cache[nn] = _build_nc(nn)
    return _nc_cache[nn]


def _numpy_fallback(X, A, W_nb, b_nb, W_line, b_line, nn):
    def leaky(x):
        return np.where(x >= 0, x, NEG_SLOPE * x)

    Ab = A > 0
    keep = Ab & (np.cumsum(Ab.astype(np.int64), axis=1) <= nn)
    cnt = keep.sum(axis=1, keepdims=True).astype(X.dtype)
    nb_sum = keep.astype(X.dtype) @ X
    nb_mean = nb_sum / np.maximum(cnt, 1.0)
    xj = leaky(nb_mean @ W_nb.T + b_nb)
    xi = leaky(X @ W_line.T + b_line)
    return (xi + np.where(cnt > 0, xj, 0.0)).astype(np.float32)


def _pack128(arr):
    """[128*k, m] -> [128, k*m] with block i in columns [i*m:(i+1)*m]."""
    k = arr.shape[0] // 128
    return np.ascontiguousarray(
        arr.reshape(k, 128, arr.shape[1]).transpose(1, 0, 2).reshape(128, -1))


def build_in_maps(X, A, W_nb, b_nb, W_line, b_line, nn):
    """Shard the full inputs into one input map per core."""
    import ml_dtypes
    bf = ml_dtypes.bfloat16
    ATall = np.ascontiguousarray((A[:, :C] > 0).T.astype(np.uint8))  # [C, N]
    XTall = np.ascontiguousarray(X.T.astype(np.float16))            # [CIN, N]
    xh = _pack128(X[:C, :].astype(bf))                               # [128, KC*CIN]
    wnbt = _pack128(np.ascontiguousarray(W_nb.T.astype(np.float32)
                                         * np.float32(1.0 / nn)).astype(bf))
    wlt = _pack128(np.ascontiguousarray(W_line.T.astype(np.float16)))
    sm = np.concatenate([np.triu(np.ones((128, 128), bf)),
                         np.ones((128, 128), bf)], axis=1)   # [128, 256]
    rcb = np.concatenate([b_nb.astype(bf).reshape(1, COUT),
                          np.ones((1, 128), bf)], axis=1)
    rcf = np.concatenate([b_line.astype(np.float16).reshape(1, COUT),
                          np.ones((1, 128), np.float16)], axis=1)
    in_maps = []
    for c in range(NCORES):
        rows = slice(c * R, (c + 1) * R)
        in_maps.append({
            "at": _pack128(ATall[:, rows]),
            "xh": xh,
            "xt": _pack128(XTall[:, rows]),
            "wnbt": wnbt,
            "wlt": wlt,
            "sm": sm,
            "rcb": rcb,
            "rcf": rcf,
        })
    return in_maps


def kernel(**inputs) -> np.ndarray:
    global LAST_RESULT
    X = np.ascontiguousarray(np.asarray(inputs["X"], dtype=np.float32))
    A = np.ascontiguousarray(np.asarray(inputs["A"], dtype=np.int32))
    W_nb = np.asarray(inputs["W_nb"], dtype=np.float32)
    b_nb = np.asarray(inputs["b_nb"], dtype=np.float32)
    W_line = np.asarray(inputs["W_line"], dtype=np.float32)
    b_line = np.asarray(inputs["b_line"], dtype=np.float32)
    nn = int(np.asarray(inputs["neibor_num"]))

    # Fast path requires: every row reaches nn set bits within the first C
    # columns (=> keep-mask confined to [:, :C] and cnt == nn > 0 per row).
    fast = (
        X.shape == (N, CIN) and A.shape == (N, N) and 1 <= nn <= C
        and int(np.count_nonzero(A[:, :C] > 0, axis=1).min()) >= nn
    )
    if not fast:
        return _numpy_fallback(X, A, W_nb, b_nb, W_line, b_line, nn)

    from concourse.bass_utils import run_bass_kernel_spmd

    in_maps = build_in_maps(X, A, W_nb, b_nb, W_line, b_line, nn)
    nc = _get_nc(nn)
    res = run_bass_kernel_spmd(nc, in_maps, core_ids=list(range(NCORES)))
    LAST_RESULT = res
    return np.concatenate([r["out"] for r in res.results], axis=0)


if __name__ == "__main__":
    rng = np.random.default_rng(0)
    X = rng.standard_normal((N, CIN), dtype=np.float32)
    A = (rng.random((N, N)) < 0.5).astype(np.int32)
    W_nb = rng.standard_normal((COUT, CIN), dtype=np.float32) * 0.04
    b_nb = rng.standard_normal(COUT, dtype=np.float32) * 0.04
    W_line = rng.standard_normal((COUT, CIN), dtype=np.float32) * 0.04
    b_line = rng.standard_normal(COUT, dtype=np.float32) * 0.04
    out = kernel(X=X, A=A, W_nb=W_nb, b_nb=b_nb, W_line=W_line,
                 b_line=b_line, neibor_num=64)
    exp = _numpy_fallback(X, A, W_nb, b_nb, W_line, b_line, 64)
    err = np.abs(out - exp).max() / np.abs(exp).max()
    print("self-test rel err:", err)


# revision 20
# speedup vs baseline: 1.2119x; 1.1055x over previous
"""Trainium2 Bass kernel for the nn_Aggregate GNN message-passing problem.

Computation (see reference):
    keep = (A > 0) limited to the first `neibor_num` set entries per row
    nb_mean = (keep @ X) / max(cnt, 1)
    out = leaky_relu(X @ W_line.T + b_line)
        + where(cnt > 0, leaky_relu(nb_mean @ W_nb.T + b_nb), 0)

Sharding: rows of A / output rows are split across 8 cores (1024 rows each).
No collectives are needed: each core gets its A row-block (transposed), its
X row-block (transposed), the shared X head rows, and the weights.

Key structural fact exploited: `keep` zeroes every set bit after the
`neibor_num`-th, so only the first C columns of A can contribute, where C
bounds the column position of the nn-th set bit over all rows.  The host
verifies exactly (cheaply) that every row reaches `neibor_num` set bits
within the first C=256 columns; in that case cnt == neibor_num for every
row and the kernel contracts over 256 neighbor candidates instead of 8192.
If the check fails (it cannot for the target input distribution), a numpy
fallback computes the exact reference semantics.

Device pipeline per core (rows R=1024, C=256, Cin=Cout=512):
  1. mask:    mbT[j, r] = (A[r, j] > 0) via DVE is_gt on the transposed
              A block (uint8 -> fp16 0/1; exact).
  2. prefix:  cumT = LTRI.T @ mbT per 128-column chunk (+ ONES.T @ mbT of
              earlier chunks) gives the inclusive prefix count of set bits
              along the row, in transposed layout, on the PE (fp16 inputs,
              fp32 accumulation; counts <= 256 so exact).
  3. keepT = (cumT <= nn) * mbT                      (one fused DVE op)
  4. nb_sumT = X_head.T-contract keepT               (PE)
  5. xj = leaky(nb_sumT.T @ (W_nb.T/nn) + b_nb)      (PE + ACT Lrelu)
     xi = leaky(X_blk @ W_line.T + b_line)           (PE + ACT Lrelu)
     out = xi + xj                                   (DVE, fp32)
Biases are added with k=1 matmuls (ones-row x bias-row) into the same PSUM
accumulation group.

Matmul operands are fp16 (e5m10): 16-bit weights enable the PE's fast
weight load / background weight buffer, so LDWEIGHTS hides behind the
matmul stream (4-byte fp32r weights serialize, ~2x slower per matmul).
Accumulation is always fp32 in PSUM; masks and counts are exact in fp16.

DMA strategy: each logical input is packed on the host into one wide
[128, *] (or [1, *]) tensor and loaded with a single DMA (descriptor issue
costs ~0.7us each, so many small loads serialize the startup).  The
latency-critical tensors (at/smalls/xh/row-consts) ride the HW DGE
(nc.sync); bulk stage-2 operands (xt, weights) ride the SW DGE (nc.gpsimd).
"""

import numpy as np

NCORES = 8
N = 8192
CIN = 512
COUT = 512
R = N // NCORES          # rows per core
C = 256                  # neighbor-candidate column window
KC = C // 128            # 128-col chunks of the window
MC = CIN // 128          # 128-row chunks of the feature dim
RT = R // 128            # 128-row output tiles per core
NEG_SLOPE = 0.01         # jax.nn.leaky_relu default

_nc_cache = {}
LAST_RESULT = None       # BassKernelResults of the most recent device run
SIM_SAFE = False         # CoreSim lacks Lrelu; True swaps in a Relu decomposition


def _build_nc(nn: int):
    import concourse.bass as bass
    import concourse.bacc as bacc
    import concourse.mybir as mybir
    import concourse.tile as tile

    F32 = mybir.dt.float32
    BF16 = mybir.dt.bfloat16  # PE fast path for the mask/xj side
    FP16 = mybir.dt.float16   # e5m10 for the precision-critical xi path
    U8 = mybir.dt.uint8
    AF = mybir.ActivationFunctionType
    OP = mybir.AluOpType

    nc = bacc.Bacc("TRN2", target_bir_lowering=False, debug=False)

    at_d = nc.dram_tensor("at", [128, KC * R], U8, kind="ExternalInput")
    xh_d = nc.dram_tensor("xh", [128, KC * CIN], BF16, kind="ExternalInput")
    xt_d = nc.dram_tensor("xt", [128, MC * R], FP16, kind="ExternalInput")
    wnbt_d = nc.dram_tensor("wnbt", [128, MC * COUT], BF16, kind="ExternalInput")
    wlt_d = nc.dram_tensor("wlt", [128, MC * COUT], FP16, kind="ExternalInput")
    sm_d = nc.dram_tensor("sm", [128, 256], BF16, kind="ExternalInput")
    rcb_d = nc.dram_tensor("rcb", [1, COUT + 128], BF16, kind="ExternalInput")
    rcf_d = nc.dram_tensor("rcf", [1, COUT + 128], FP16, kind="ExternalInput")
    out_d = nc.dram_tensor("out", [R, COUT], F32, kind="ExternalOutput")

    with tile.TileContext(nc) as tc:
        with (
            tc.tile_pool(name="const", bufs=1) as constp,
            tc.tile_pool(name="mask", bufs=1) as maskp,
            tc.tile_pool(name="work", bufs=3) as workp,
            tc.tile_pool(name="psum", bufs=2, space=bass.MemorySpace.PSUM) as psump,
        ):
            # --- latency-critical loads (SW DGE: aggregates the 2KB lines
            # of these small tensors into 16KB packets) ------------------
            at_sb = maskp.tile([128, KC * R], U8, name="at_sb")
            nc.gpsimd.dma_start(at_sb[:], at_d[:])
            sm = constp.tile([128, 256], BF16, name="sm_sb")
            nc.gpsimd.dma_start(sm[:], sm_d[:])
            xh_sb = constp.tile([128, KC * CIN], BF16, name="xh_sb")
            nc.gpsimd.dma_start(xh_sb[:], xh_d[:])
            rcb = constp.tile([1, COUT + 128], BF16, name="rcb_sb")
            nc.gpsimd.dma_start(rcb[:], rcb_d[:])
            rcf = constp.tile([1, COUT + 128], FP16, name="rcf_sb")
            nc.gpsimd.dma_start(rcf[:], rcf_d[:])

            ltri = sm[:, 0:128]
            ones = sm[:, 128:256]
            bnb = rcb[:, 0:COUT]
            onesb = rcb[:, COUT:]
            bl = rcf[:, 0:COUT]
            onesf = rcf[:, COUT:]
            at = [at_sb[:, t * R:(t + 1) * R] for t in range(KC)]
            xh = [xh_sb[:, t * CIN:(t + 1) * CIN] for t in range(KC)]

            # --- bulk stage-2 operands (HW DGE; 4-8KB lines) -------------
            xt_sb = constp.tile([128, MC * R], FP16, name="xt_sb")
            nc.sync.dma_start(xt_sb[:], xt_d[:])
            wlt_sb = constp.tile([128, MC * COUT], FP16, name="wlt_sb")
            nc.sync.dma_start(wlt_sb[:], wlt_d[:])
            wnbt_sb = constp.tile([128, MC * COUT], BF16, name="wnbt_sb")
            nc.sync.dma_start(wnbt_sb[:], wnbt_d[:])
            xt = [xt_sb[:, m * R:(m + 1) * R] for m in range(MC)]
            wnbt = [wnbt_sb[:, m * COUT:(m + 1) * COUT] for m in range(MC)]
            wlt = [wlt_sb[:, m * COUT:(m + 1) * COUT] for m in range(MC)]

            # 1. A block -> fp16 0/1 mask, transposed layout [col, row]
            mb = []
            for t in range(KC):
                mb_t = maskp.tile([128, R], BF16, name=f"mb{t}")
                nc.vector.tensor_scalar(mb_t[:], at[t], 0, None, op0=OP.is_gt)
                mb.append(mb_t)

            # 2+3. prefix count along the row (PE) -> keep mask (DVE)
            keep = []
            for t in range(KC):
                keep_t = maskp.tile([128, R], BF16, name=f"keep{t}")
                keep.append(keep_t)
            for t in range(KC):
                for h in range(R // 512):
                    sl = slice(h * 512, (h + 1) * 512)
                    cum = psump.tile([128, 512], F32, name="cum")
                    for s in range(t + 1):
                        nc.tensor.matmul(
                            cum[:],
                            ltri if s == t else ones,
                            mb[s][:, sl],
                            start=(s == 0),
                            stop=(s == t),
                        )
                    # keep = (cum <= nn) * mb
                    nc.vector.scalar_tensor_tensor(
                        keep[t][:, sl], cum[:], float(nn), mb[t][:, sl],
                        op0=OP.is_le, op1=OP.mult,
                    )

            # 4. nb_sumT[c, r] = sum_k X[k, c] * keep[k, r]  (the pre-scaled
            #    weights absorb the 1/nn mean factor)
            nbm = []
            for m in range(MC):
                nbm_m = maskp.tile([128, R], BF16, name=f"nbm{m}")
                nbm.append(nbm_m)
            for m in range(MC):
                for h in range(R // 512):
                    sl = slice(h * 512, (h + 1) * 512)
                    ps = psump.tile([128, 512], F32, name="psnb")
                    for t in range(KC):
                        nc.tensor.matmul(
                            ps[:],
                            xh[t][:, m * 128:(m + 1) * 128],
                            keep[t][:, sl],
                            start=(t == 0),
                            stop=(t == KC - 1),
                        )
                    # PSUM -> SBUF copies split between ACT and DVE
                    if (m * 2 + h) % 2 == 0:
                        nc.scalar.activation(nbm[m][:, sl], ps[:], AF.Copy)
                    else:
                        nc.vector.tensor_copy(nbm[m][:, sl], ps[:])

            # 5. two linears + leaky relu + add, per 128-row output tile
            def leaky(ps, out_sb):
                if SIM_SAFE:
                    t = workp.tile([128, COUT], F32, name="lrt")
                    nc.scalar.activation(t[:], ps[:], AF.Relu,
                                         scale=1.0 - NEG_SLOPE)
                    nc.vector.scalar_tensor_tensor(
                        out_sb[:], ps[:], NEG_SLOPE, t[:],
                        op0=OP.mult, op1=OP.add)
                else:
                    nc.scalar.activation(out_sb[:], ps[:], AF.Lrelu,
                                         alpha=NEG_SLOPE)

            for r in range(RT):
                rsl = slice(r * 128, (r + 1) * 128)
                psi = psump.tile([128, COUT], F32, name="psi")
                for m in range(MC):
                    nc.tensor.matmul(
                        psi[:], xt[m][:, rsl], wlt[m],
                        start=(m == 0), stop=False,
                    )
                nc.tensor.matmul(psi[:], onesf, bl, start=False, stop=True)
                xi = workp.tile([128, COUT], F32, name="xi")
                leaky(psi, xi)

                psj = psump.tile([128, COUT], F32, name="psj")
                for m in range(MC):
                    nc.tensor.matmul(
                        psj[:], nbm[m][:, rsl], wnbt[m],
                        start=(m == 0), stop=False,
                    )
                nc.tensor.matmul(psj[:], onesb, bnb, start=False, stop=True)
                xj = workp.tile([128, COUT], F32, name="xj")
                leaky(psj, xj)

                ot = workp.tile([128, COUT], F32, name="ot")
                nc.vector.tensor_tensor(ot[:], xi[:], xj[:], op=OP.add)
                eng = nc.sync if r % 2 == 0 else nc.scalar
                eng.dma_start(out_d[rsl, :], ot[:])

    nc.compile()
    return nc


def _get_nc(nn: int):
    if nn not in _nc_cache:
        _nc_cache[nn] = _build_nc(nn)
    return _nc_cache[nn]


def _numpy_fallback(X, A, W_nb, b_nb, W_line, b_line, nn):
    def leaky(x):
        return np.where(x >= 0, x, NEG_SLOPE * x)

    Ab = A > 0
    keep = Ab & (np.cumsum(Ab.astype(np.int64), axis=1) <= nn)
    cnt = keep.sum(axis=1, keepdims=True).astype(X.dtype)
    nb_sum = keep.astype(X.dtype) @ X
    nb_mean = nb_sum / np.maximum(cnt, 1.0)
    xj = leaky(nb_mean @ W_nb.T + b_nb)
    xi = leaky(X @ W_line.T + b_line)
    return (xi + np.where(cnt > 0, xj, 0.0)).astype(np.float32)


def _pack128(arr):
    """[128*k, m] -> [128, k*m] with block i in columns [i*m:(i+1)*m]."""
    k = arr.shape[0] // 128
    return np.ascontiguousarray(
        arr.reshape(k, 128, arr.shape[1]).transpose(1, 0, 2).reshape(128, -1))


def build_in_maps(X, A, W_nb, b_nb, W_line, b_line, nn):
    """Shard the full inputs into one input map per core."""
    import ml_dtypes
    bf = ml_dtypes.bfloat16
    ATall = np.ascontiguousarray((A[:, :C] > 0).T.astype(np.uint8))  # [C, N]
    XTall = np.ascontiguousarray(X.T.astype(np.float16))            # [CIN, N]
    xh = _pack128(X[:C, :].astype(bf))                               # [128, KC*CIN]
    wnbt = _pack128(np.ascontiguousarray(W_nb.T.astype(np.float32)
                                         * np.float32(1.0 / nn)).astype(bf))
    wlt = _pack128(np.ascontiguousarray(W_line.T.astype(np.float16)))
    sm = np.concatenate([np.triu(np.ones((128, 128), bf)),
                         np.ones((128, 128), bf)], axis=1)   # [128, 256]
    rcb = np.concatenate([b_nb.astype(bf).reshape(1, COUT),
                          np.ones((1, 128), bf)], axis=1)
    rcf = np.concatenate([b_line.astype(np.float16).reshape(1, COUT),
                          np.ones((1, 128), np.float16)], axis=1)
    in_maps = []
    for c in range(NCORES):
        rows = slice(c * R, (c + 1) * R)
        in_maps.append({
            "at": _pack128(ATall[:, rows]),
            "xh": xh,
            "xt": _pack128(XTall[:, rows]),
            "wnbt": wnbt,
            "wlt": wlt,
            "sm": sm,
            "rcb": rcb,
            "rcf": rcf,
        })
    return in_maps


def kernel(**inputs) -> np.ndarray:
    global LAST_RESULT
    X = np.ascontiguousarray(np.asarray(inputs["X"], dtype=np.float32))
    A = np.ascontiguousarray(np.asarray(inputs["A"], dtype=np.int32))
    W_nb = np.asarray(inputs["W_nb"], dtype=np.float32)
    b_nb = np.asarray(inputs["b_nb"], dtype=np.float32)
    W_line = np.asarray(inputs["W_line"], dtype=np.float32)
    b_line = np.asarray(inputs["b_line"], dtype=np.float32)
    nn = int(np.asarray(inputs["neibor_num"]))

    # Fast path requires: every row reaches nn set bits within the first C
    # columns (=> keep-mask confined to [:, :C] and cnt == nn > 0 per row).
    fast = (
        X.shape == (N, CIN) and A.shape == (N, N) and 1 <= nn <= C
        and int(np.count_nonzero(A[:, :C] > 0, axis=1).min()) >= nn
    )
    if not fast:
        return _numpy_fallback(X, A, W_nb, b_nb, W_line, b_line, nn)

    from concourse.bass_utils import run_bass_kernel_spmd

    in_maps = build_in_maps(X, A, W_nb, b_nb, W_line, b_line, nn)
    nc = _get_nc(nn)
    res = run_bass_kernel_spmd(nc, in_maps, core_ids=list(range(NCORES)))
    LAST_RESULT = res
    return np.concatenate([r["out"] for r in res.results], axis=0)


if __name__ == "__main__":
    rng = np.random.default_rng(0)
    X = rng.standard_normal((N, CIN), dtype=np.float32)
    A = (rng.random((N, N)) < 0.5).astype(np.int32)
    W_nb = rng.standard_normal((COUT, CIN), dtype=np.float32) * 0.04
    b_nb = rng.standard_normal(COUT, dtype=np.float32) * 0.04
    W_line = rng.standard_normal((COUT, CIN), dtype=np.float32) * 0.04
    b_line = rng.standard_normal(COUT, dtype=np.float32) * 0.04
    out = kernel(X=X, A=A, W_nb=W_nb, b_nb=b_nb, W_line=W_line,
                 b_line=b_line, neibor_num=64)
    exp = _numpy_fallback(X, A, W_nb, b_nb, W_line, b_line, 64)
    err = np.abs(out - exp).max() / np.abs(exp).max()
    print("self-test rel err:", err)
